# revision 29
# baseline (speedup 1.0000x reference)
"""GQA kernel for trn2: 8 NeuronCores, SPMD (b in {0,1} x 4 head-groups).

Per core (b, hg): 8 q-heads, 2 kv-heads. All matmuls in bf16 (f32r runs
at 4 cyc/row on HW; bf16 at 1). Head pairing (m, m+4) puts the kv0-head
at partitions 0-63 and the kv1-head at 64-127 of each q tile, so score
matmuls read k directly (no kswap) and auto row-tile at (0,0)/(64,0),
running concurrently in the PE array. Pipeline order per chunk:
attn(c) -> proj(c+1) -> oproj(c), so projection matmuls hide the
softmax-normalize latency of the last head. Partial outputs summed on
host (row-parallel Wo all-reduce).
"""
import numpy as np
import ml_dtypes
import concourse.bass as bass
import concourse.mybir as mybir
from concourse import tile, bacc
from concourse.bass_utils import run_bass_kernel_spmd

B, S, D = 2, 2048, 2048
H, KVH, DH = 32, 8, 64
SCALE = DH ** -0.5
KD = 16         # D contraction chunks of 128
F32 = mybir.dt.float32
BF16 = mybir.dt.bfloat16

_cache = {}


def build():
    nc = bacc.Bacc('TRN2', target_bir_lowering=False, debug=False, num_devices=8)
    xT_p = nc.declare_dram_parameter('xT', [D, S], BF16, isOutput=False)
    WT_p = nc.declare_dram_parameter('WT', [D, 768], BF16, isOutput=False)
    WoT_p = nc.declare_dram_parameter('WoT', [512, D], BF16, isOutput=False)
    cos4_p = nc.declare_dram_parameter('cos4', [128, S], BF16, isOutput=False)
    sin4_p = nc.declare_dram_parameter('sin4', [128, S], BF16, isOutput=False)
    mask_p = nc.declare_dram_parameter('mask', [128, 4 * 1024], BF16, isOutput=False)
    ident_p = nc.declare_dram_parameter('ident', [128, 128], BF16, isOutput=False)
    out_p = nc.declare_dram_parameter('out', [S, D], BF16, isOutput=True)

    with tile.TileContext(nc) as tc:
        with tc.tile_pool(name='w', bufs=1) as wpool, \
             tc.tile_pool(name='x', bufs=32) as xpool, \
             tc.tile_pool(name='qk', bufs=1) as qkpool, \
             tc.tile_pool(name='tmp', bufs=3) as tpool, \
             tc.tile_pool(name='at', bufs=4) as atpool, \
             tc.tile_pool(name='ost', bufs=2) as ostpool, \
             tc.tile_pool(name='big', bufs=3, space='PSUM') as pbig, \
             tc.tile_pool(name='sm', bufs=1, space='PSUM') as psm:

            # interleave WT slices with the first chunk's x tiles so the
            # first projection matmul starts ~3us in, not after 7MB of DMA
            WT = wpool.tile([128, KD * 768], BF16, tag='WT', name='WT')
            xts0 = []
            for kd in range(KD):
                nc.sync.dma_start(out=WT[:, kd * 768:(kd + 1) * 768],
                                  in_=WT_p[128 * kd:128 * (kd + 1), :])
                xt = xpool.tile([128, 512], BF16, tag='xt')
                nc.sync.dma_start(out=xt[:], in_=xT_p[128 * kd:128 * (kd + 1), 0:512])
                xts0.append(xt)
            cos4 = wpool.tile([128, S], BF16, tag='cos4')
            sin4 = wpool.tile([128, S], BF16, tag='sin4')
            masks = wpool.tile([128, 4 * 1024], BF16, tag='masks')
            ident = wpool.tile([128, 128], BF16, tag='ident')
            nc.sync.dma_start(out=cos4[:], in_=cos4_p[:])
            nc.sync.dma_start(out=sin4[:], in_=sin4_p[:])
            nc.sync.dma_start(out=masks[:], in_=mask_p[:])
            nc.sync.dma_start(out=ident[:], in_=ident_p[:])
            WoT = wpool.tile([128, 4 * D], BF16, tag='WoT')
            for hc in range(4):
                nc.sync.dma_start(out=WoT[:, hc * D:(hc + 1) * D],
                                  in_=WoT_p[128 * hc:128 * (hc + 1), :])

            # persistent SBUF tensors
            qk = [qkpool.tile([128, S], BF16, tag=f'qk{m}', name=f'qk{m}')
                  for m in range(5)]
            vT = qkpool.tile([128, S], BF16, tag='vT')
            V = qkpool.tile([128, 16 * 130], BF16, tag='V')
            aout = [qkpool.tile([128, S], BF16, tag=f'ao{i}', name=f'ao{i}')
                    for i in range(4)]

            def rope(ps_half, m, s):
                # qk[m][:, 512s:512(s+1)] = ps*cos4 + swap32(ps)*sin4
                b16 = tpool.tile([128, 512], BF16, tag='b16')
                bsw = tpool.tile([128, 512], BF16, tag='bsw')
                t1 = tpool.tile([128, 512], BF16, tag='t1')
                t2 = tpool.tile([128, 512], BF16, tag='t2')
                nc.scalar.copy(b16[:], ps_half)
                for b0 in (0, 64):
                    nc.vector.tensor_copy(bsw[b0:b0 + 32, :], b16[b0 + 32:b0 + 64, :])
                    nc.vector.tensor_copy(bsw[b0 + 32:b0 + 64, :], b16[b0:b0 + 32, :])
                nc.vector.tensor_mul(t1[:], b16[:], cos4[:, 512 * s:512 * (s + 1)])
                nc.vector.tensor_mul(t2[:], bsw[:], sin4[:, 512 * s:512 * (s + 1)])
                nc.vector.tensor_add(qk[m][:, 512 * s:512 * (s + 1)], t1[:], t2[:])

            def proj_mg(s, xts, mg):
                ps = pbig.tile([128, 1024], F32, tag='big')
                if mg < 2:
                    for half, m in enumerate((2 * mg, 2 * mg + 1)):
                        sl = ps[:, 512 * half:512 * half + 512]
                        for kd in range(KD):
                            nc.tensor.matmul(
                                sl[:],
                                WT[:, kd * 768 + 128 * m:kd * 768 + 128 * (m + 1)],
                                xts[kd][:], start=(kd == 0), stop=(kd == KD - 1))
                        rope(sl[:], m, s)
                else:
                    # k pair -> cols 0:512 (rope), v pair -> cols 512:1024
                    for kd in range(KD):
                        nc.tensor.matmul(
                            ps[:, 0:512],
                            WT[:, kd * 768 + 512:kd * 768 + 640],
                            xts[kd][:], start=(kd == 0), stop=(kd == KD - 1))
                    for kd in range(KD):
                        nc.tensor.matmul(
                            ps[:, 512:1024],
                            WT[:, kd * 768 + 640:kd * 768 + 768],
                            xts[kd][:], start=(kd == 0), stop=(kd == KD - 1))
                    rope(ps[:, 0:512], 4, s)
                    nc.scalar.copy(vT[:, 512 * s:512 * (s + 1)], ps[:, 512:1024])

            def v_setup(s):
                # V natural (ones-augmented): per block b: [v0 64|1|v1 64|1]
                for b in range(4 * s, 4 * s + 4):
                    pt = pbig.tile([128, 128], BF16, tag='big', name='pt')
                    nc.tensor.transpose(pt[:], vT[:, 128 * b:128 * (b + 1)],
                                        ident[:])
                    nc.vector.tensor_copy(V[:, 130 * b:130 * b + 64], pt[:, 0:64])
                    nc.vector.tensor_copy(V[:, 130 * b + 65:130 * b + 129],
                                          pt[:, 64:128])
                    nc.vector.memset(V[:, 130 * b + 64:130 * b + 65], 1.0)
                    nc.vector.memset(V[:, 130 * b + 129:130 * b + 130], 1.0)

            for mg in (2, 0, 1):
                proj_mg(0, xts0, mg)
            v_setup(0)
            for c in range(4):
                # prefetch next chunk's x tiles
                if c < 3:
                    xts = []
                    for kd in range(KD):
                        xt = xpool.tile([128, 512], BF16, tag='xt')
                        nc.sync.dma_start(
                            out=xt[:],
                            in_=xT_p[128 * kd:128 * (kd + 1),
                                     512 * (c + 1):512 * (c + 2)])
                        xts.append(xt)
                # ---- attention for query chunk c, interleaved with the
                # next chunk's projections (fills ACT-bound tensor idle) ----
                nj = 4 * c + 4
                for hp in range(4):
                    oAB = psm.tile([65, 1024], F32, tag='oAB')
                    oA = oAB[:, 0:512]
                    oB = oAB[:, 512:1024]
                    for j in range(nj):
                        pair = pbig.tile([128, 1024], F32, tag='big', name='pair')
                        nc.tensor.matmul(pair[:, 0:512],
                                         qk[4][0:64, 128 * j:128 * (j + 1)],
                                         qk[hp][0:64, 512 * c:512 * (c + 1)],
                                         start=True, stop=True)
                        nc.tensor.matmul(pair[:, 512:1024],
                                         qk[4][64:128, 128 * j:128 * (j + 1)],
                                         qk[hp][64:128, 512 * c:512 * (c + 1)],
                                         start=True, stop=True)
                        ata = atpool.tile([128, 1024], BF16, tag='at')
                        nc.scalar.activation(ata[:], pair[:],
                                             mybir.ActivationFunctionType.Exp,
                                             scale=SCALE)
                        d = j - 4 * c
                        if 0 <= d <= 3:
                            nc.vector.tensor_mul(ata[:], ata[:],
                                                 masks[:, 1024 * d:1024 * (d + 1)])
                        nc.tensor.matmul(oA, V[:, 130 * j:130 * j + 65],
                                         ata[:, 0:512],
                                         start=(j == 0), stop=(j == nj - 1))
                        nc.tensor.matmul(oB, V[:, 130 * j + 65:130 * j + 130],
                                         ata[:, 512:1024],
                                         start=(j == 0), stop=(j == nj - 1))
                    # copy oA/oB out of PSUM immediately (releases the banks
                    # for the next hp), then normalize from the SBUF copies
                    den2 = tpool.tile([1, 1024], F32, tag='den2')
                    nc.vector.tensor_copy(den2[:], oAB[64:65, :])
                    oc = tpool.tile([64, 1024], BF16, tag='oc')
                    nc.vector.tensor_copy(oc[:], oAB[0:64, :])
                    rA = tpool.tile([1, 512], F32, tag='rA')
                    rB = tpool.tile([1, 512], F32, tag='rB')
                    nc.vector.reciprocal_approx_fast(out=rA[:], in_=den2[:, 0:512])
                    nc.vector.reciprocal_approx_fast(out=rB[:], in_=den2[:, 512:1024])
                    rA16 = tpool.tile([1, 512], BF16, tag='rA16')
                    rB16 = tpool.tile([1, 512], BF16, tag='rB16')
                    nc.vector.tensor_copy(rA16[:], rA[:])
                    nc.vector.tensor_copy(rB16[:], rB[:])
                    bcA = tpool.tile([64, 512], BF16, tag='bcA')
                    bcB = tpool.tile([64, 512], BF16, tag='bcB')
                    nc.gpsimd.partition_broadcast(bcA[:], rA16[0:1, :])
                    nc.gpsimd.partition_broadcast(bcB[:], rB16[0:1, :])
                    nc.vector.tensor_mul(aout[hp][0:64, 512 * c:512 * (c + 1)],
                                         oc[:, 0:512], bcA[:])
                    nc.vector.tensor_mul(aout[hp][64:128, 512 * c:512 * (c + 1)],
                                         oc[:, 512:1024], bcB[:])

                # ---- projections for the next chunk (hides normalize tail) ----
                if c < 3:
                    for mg in (2, 0, 1):
                        proj_mg(c + 1, xts, mg)
                    v_setup(c + 1)

                # ---- output projection for token blocks of chunk c ----
                for sb in range(4 * c, 4 * c + 4):
                    ost = ostpool.tile([128, D], BF16, tag='ost')
                    for dg in range(2):
                        po = pbig.tile([128, 1024], F32, tag='big', name='po')
                        for hc in range(4):
                            for dc in (2 * dg, 2 * dg + 1):
                                nc.tensor.matmul(
                                    po[:, 512 * (dc - 2 * dg):512 * (dc - 2 * dg) + 512],
                                    aout[hc][:, 128 * sb:128 * (sb + 1)],
                                    WoT[:, hc * D + 512 * dc:hc * D + 512 * (dc + 1)],
                                    start=(hc == 0), stop=(hc == 3))
                        nc.scalar.copy(ost[:, 1024 * dg:1024 * (dg + 1)], po[:])
                    nc.sync.dma_start(out=out_p[128 * sb:128 * (sb + 1), :], in_=ost[:])
    nc.compile()
    return nc


_PERM = np.concatenate([np.arange(0, DH, 2), np.arange(1, DH, 2)])
_PAIR = [0, 4, 1, 5, 2, 6, 3, 7]


def _prep_core(x, Wq, Wk, Wv, Wo, cos, sin, b, hg):
    bf = ml_dtypes.bfloat16
    xT = np.ascontiguousarray(x[b].T).astype(bf)
    # q heads paired (m, m+4), each RoPE-permuted; k heads 2hg,2hg+1 permuted;
    # v heads natural
    wq_l = Wq.reshape(H, DH, D)[8 * hg:8 * hg + 8][:, _PERM, :]
    wq = wq_l[_PAIR].reshape(512, D)
    wk = Wk.reshape(KVH, DH, D)[2 * hg:2 * hg + 2][:, _PERM, :].reshape(128, D)
    wv = Wv.reshape(KVH, DH, D)[2 * hg:2 * hg + 2].reshape(128, D)
    WT = np.ascontiguousarray(np.concatenate([wq, wk, wv], 0).T).astype(bf)
    # WoT rows ordered to match aout partition layout (pairs (hp, hp+4))
    wo_l = Wo[:, 512 * hg:512 * (hg + 1)].T.reshape(8, DH, D)
    WoT = np.ascontiguousarray(wo_l[_PAIR].reshape(512, D)).astype(bf)
    cosT = cos.T.astype(np.float32)          # (32, S)
    sinT = sin.T.astype(np.float32)
    cos4 = np.tile(cosT, (4, 1)).astype(bf)
    sin4 = np.concatenate([-sinT, sinT, -sinT, sinT], 0).astype(bf)
    mask = np.zeros((128, 4 * 1024), dtype=np.float64)
    for dd in range(4):
        tri = (128 * dd + np.arange(128)[:, None]) <= np.arange(512)[None, :]
        mask[:, 1024 * dd:1024 * dd + 512] = tri
        mask[:, 1024 * dd + 512:1024 * (dd + 1)] = tri
    return {'xT': xT, 'WT': WT, 'WoT': WoT, 'cos4': cos4, 'sin4': sin4,
            'mask': mask.astype(bf), 'ident': np.eye(128, dtype=bf)}


def _run(inputs, trace=False, tmpdir=None):
    if 'nc' not in _cache:
        _cache['nc'] = build()
    in_maps = [_prep_core(inputs['x'], inputs['Wq'], inputs['Wk'], inputs['Wv'],
                          inputs['Wo'], inputs['cos'], inputs['sin'], c // 4, c % 4)
               for c in range(8)]
    res = run_bass_kernel_spmd(_cache['nc'], in_maps, core_ids=list(range(8)),
                               trace=trace, tmpdir=tmpdir)
    parts = [res.results[c]['out'].astype(np.float32) for c in range(8)]
    out = np.stack([parts[0] + parts[1] + parts[2] + parts[3],
                    parts[4] + parts[5] + parts[6] + parts[7]], 0)
    return out.astype(np.float32), res


def kernel(**inputs):
    out, _ = _run(inputs, trace=False)
    return out
